# revision 28
# baseline (speedup 1.0000x reference)
"""Trainium2 Bass kernel for nn_MeshTransformer (8-core SPMD, V-sharded).

Computes, for each of BS=256 (b,s) pairs:
    out[bs, v, i] = sum_{p,j} ws[bs,p] * R[i,j](bs,p) * deformed[p,v,j]
                    + sum_p w[bs,p] * t[bs,p,i]
with R the XYZ-euler rotation, ws = w * scale, deformed = base + offsets.

Mapping (vs. the fp16 predecessor: 4x less PE work via fp8 DoubleRow):
  - Vertex dim V (2562, padded to 2568) is sharded 8 ways (321/core).
  - The whole per-(bs,v,i) contraction runs as ONE fp8e4m3 DoubleRow matmul
    per (i-plane, bs-half) group: 206 contraction slots packed 2-per-partition
    into K=103:
      * 192 slots: ws*R[i,j] x offsets[p,v,j]   (fp8 error only touches the
        small offsets term, |O| ~ 0.1)
      * 12 slots: hi/lo error-compensated split of A[i,j]=sum_p ws*R (lhs) x
        base_verts[v,j] (rhs) - the large rank-3 base term at ~fp16 accuracy
      * 2 slots: hi/lo split of the translation sum against a ones-row
    Host does all weight math in fp64 and ships ready fp8 bytes; matmul
    abs err ~0.14 vs absmax 34 (rel ~4e-3 with the uint8 output, budget 2e-2).
  - 6 groups x 2 N-chunks (161+160 <= 256 moving limit) = 12 matmuls,
    963 PE cycles total at 0.5 cycles/row; queue-stuffed so dispatch lands
    past the 3us p-state ramp and runs at the full 2.4 GHz clock.
  - Input: [D | W0 | W1] on SP HWDGE plus [W2..W5] on Pool direct SWDGE
    (desc-gen on the idle Pool engine; its transfer queues right behind
    chunk 1). Only the 103 data-carrying partition rows are moved.
  - Drains quantize PSUM->uint8 (out*QS+128) on ACT & DVE: an early single
    each (DVE, the slower engine, takes g0, the earliest), then a two-bank
    pair each ([128,2,321] in one op), balanced to finish within ~20ns.
  - Output: two SP HWDGE copies (singles' columns early, pair columns after
    the last drain) with engine-drain sem handshakes; host dequantizes.
"""

import numpy as np
from contextlib import ExitStack

import concourse.bass as bass
import concourse.tile as tile
from concourse import bacc, mybir
from concourse.bass_utils import run_bass_kernel_spmd

B, S, P, V = 16, 16, 64, 2562
BS = B * S              # 256
N_CORES = 8
VPAD = 2568             # minimal multiple of 8 covering V
VC = VPAD // N_CORES    # 321 vertices per core

K = 103                 # contraction partitions (206 slots / 2 per partition)
NSLOT = 206
T1COLS = 592            # chunk1 per-k-tile cols: 321 D | W0 W1 | pad (16B-aligned stride for dual-fp8 ldweights)
T2COLS = 512            # chunk2 per-k-tile cols: W2..W5
LINE1 = 2 * T1COLS      # 1280 B/row
LINE2 = 2 * T2COLS      # 1024 B/row
OUTC = 6 * VC           # 1926 osb columns
QS = 3.6                # uint8 output quantization scale (|out| <= ~35)

F32 = mybir.dt.float32
F16 = mybir.dt.float16
F8 = mybir.dt.float8e4
U8 = mybir.dt.uint8

GROUPS = [(0, 0), (1, 0), (2, 0), (0, 1), (1, 1), (2, 1)]  # (i, h) per g


def _build_kernel():
    nc = bacc.Bacc("TRN2", target_bir_lowering=False, debug=False)

    in1_d = nc.dram_tensor("in1", [128, LINE1], U8, kind="ExternalInput").ap()
    in2_d = nc.dram_tensor("in2", [128, LINE2], U8, kind="ExternalInput").ap()
    out_d = nc.dram_tensor("out", [128, OUTC], U8, kind="ExternalOutput").ap()

    with tile.TileContext(nc) as tc, ExitStack() as ctx:
        pool = ctx.enter_context(tc.tile_pool(name="work", bufs=1))
        psum = ctx.enter_context(tc.tile_pool(name="psum", bufs=1, space="PSUM"))
        psumw = ctx.enter_context(tc.tile_pool(name="psumw", bufs=1, space="PSUM"))

        # engine warmups: ACT function-table preload + PE p-state warm-up;
        # the bias tile feeds the ACT drains (quantization offset +128)
        dummy = pool.tile([128, 1], F16, tag="dummy")
        biasq = pool.tile([128, 1], F32, tag="biasq")
        nc.vector.memset(dummy[:], 0.25)

        # input chunk 1 (D + W0..W2) on SP HWDGE; chunk 2 (W3..W5) via Pool
        # SWDGE - its desc-gen runs on the otherwise-idle Pool engine and its
        # transfer queues right behind chunk 1, arriving ~140ns earlier than a
        # second serialized SP HWDGE chain would
        t1 = pool.tile([128, 2, T1COLS], F8, tag="t1")
        t2 = pool.tile([128, 2, T2COLS], F8, tag="t2")
        # only rows 0..K-1 carry data - don't move the zero padding rows
        nc.sync.dma_start(
            out=t1[:].bitcast(U8).rearrange("p a b -> p (a b)")[0:K, :],
            in_=in1_d[0:K, :])
        nc.gpsimd.dma_start(
            out=t2[:].bitcast(U8).rearrange("p a b -> p (a b)")[0:K, :],
            in_=in2_d[0:K, :])

        nc.gpsimd.memset(biasq[:], 128.0)
        nc.scalar.copy(dummy[:], biasq[:])
        wps = psumw.tile([1, 1], F32)
        nc.tensor.matmul(wps[:], dummy[:], dummy[:], start=True, stop=True)

        # osb lives at a fixed SBUF address under two names: drains write
        # osb (tracked), the output DMAs read osbr (untracked alias) so tile
        # adds no redundant split-wait EventSemaphores on SP - the real sync
        # is the single ds1/ds2 drain-handshake wait per DMA.
        arena = ctx.enter_context(nc.sbuf_tensor([128, OUTC], U8))
        addr = nc.lookup_mloc(arena).addr
        osb = nc.alloc_sbuf_tensor_at("osbw", [128, OUTC], U8, offset=addr).ap()
        osbr = nc.alloc_sbuf_tensor_at("osbr", [128, OUTC], U8, offset=addr).ap()

        # stuff the PE wait queue with no-op matmuls gated on t1 so the real
        # matmuls' seq dispatch (where the p-state is sampled) happens with the
        # sequencer continuously busy -> full clock past the ramp window
        for _ in range(2):
            nc.tensor.matmul(wps[:], t1[0:K, 0, 0:1], t1[0:K, 0, 0:1],
                             start=True, stop=True)

        # 6 groups x 2 N-chunks of fp8 DoubleRow matmuls (full contraction per
        # instruction: start=stop=True, disjoint PSUM column ranges).
        # psum tile m holds groups (2m, 2m+1) in its two banks so a pair can
        # drain as one [128, 2, VC] op.
        DR = mybir.MatmulPerfMode.DoubleRow

        def w_ap(g):
            if g < 2:
                return t1[0:K, :, VC + g * 128:VC + (g + 1) * 128]
            return t2[0:K, :, (g - 2) * 128:(g - 1) * 128]

        # separate psum tiles per drain unit: two singles (g0, g1) and two
        # 2-bank pairs ((g2,g3), (g4,g5)); a shared tile between two drains
        # makes tile-tracking serialize them cross-engine
        ptA = psum.tile([128, 512], F32, tag="ptA")
        ptB = psum.tile([128, 512], F32, tag="ptB")
        ptCD = psum.tile([128, 2, 512], F32, tag="ptCD")
        ptEF = psum.tile([128, 2, 512], F32, tag="ptEF")
        def ps_ap(g, c0, c1):
            if g == 0: return ptA[:, c0:c1]
            if g == 1: return ptB[:, c0:c1]
            pt = ptCD if g < 4 else ptEF
            return pt[:, g % 2, c0:c1]
        for g in range(6):
            for c0, c1 in ((0, 161), (161, VC)):
                nc.tensor.matmul(ps_ap(g, c0, c1), w_ap(g),
                                 t1[0:K, :, c0:c1],
                                 start=True, stop=True, perf_mode=DR)

        # drains: uint8 = psum*QS + 128; singles land first, then the pairs;
        # ACT (faster) takes the last-finishing pair
        ALU = mybir.AluOpType
        IDT = mybir.ActivationFunctionType.Identity
        ds1 = nc.alloc_semaphore("ds1")
        ds2 = nc.alloc_semaphore("ds2")
        nc.vector.tensor_scalar(osb[:, 0:VC], ptA[:, 0:VC],
                                float(QS), 128.0, op0=ALU.mult, op1=ALU.add)
        nc.scalar.activation(osb[:, VC:2 * VC], ptB[:, 0:VC], IDT,
                             bias=biasq[:], scale=float(QS))
        # engine-drain handshakes give the output DMAs a hardware-enforced
        # completion signal (belt and suspenders vs engine-tick sems)
        nc.scalar.drain().then_inc(ds1, 1)
        nc.vector.drain().then_inc(ds1, 1)
        nc.vector.tensor_scalar(osb[:, 2 * VC:4 * VC], ptCD[:, :, 0:VC],
                                float(QS), 128.0, op0=ALU.mult, op1=ALU.add)
        nc.scalar.activation(osb[:, 4 * VC:6 * VC], ptEF[:, :, 0:VC], IDT,
                             bias=biasq[:], scale=float(QS))
        nc.scalar.drain().then_inc(ds2, 1)
        nc.vector.drain().then_inc(ds2, 1)

        # output in two pieces: the singles' columns fire early, the rest
        # after the pair drains land; host dequantizes
        nc.sync.dma_start(
            out=out_d[:, 0:2 * VC], in_=osbr[:, 0:2 * VC])._wait_ge(ds1, 2)
        nc.sync.dma_start(
            out=out_d[:, 2 * VC:6 * VC],
            in_=osbr[:, 2 * VC:6 * VC])._wait_ge(ds2, 2)

    nc.compile()
    return nc


_NC_CACHE = None


def _get_nc():
    global _NC_CACHE
    if _NC_CACHE is None:
        _NC_CACHE = _build_kernel()
    return _NC_CACHE


def _prep_inputs(scales, transforms, prototype_weights, prototype_offsets, base_verts):
    """Host-side math (fp64) + fp8 packing; device is pure DMA+PE+drain."""
    f8np = mybir.dt.np(F8)

    def q8(x):
        return np.asarray(x, np.float32).astype(f8np)

    f = np.float64
    scl = np.asarray(scales, np.float32).reshape(BS).astype(f)
    tf = np.asarray(transforms, np.float32).reshape(BS, P, 6).astype(f)
    w = np.asarray(prototype_weights, np.float32).reshape(BS, P).astype(f)
    t = tf[:, :, 0:3]
    sa, ca = np.sin(tf[:, :, 3]), np.cos(tf[:, :, 3])
    sb, cb = np.sin(tf[:, :, 4]), np.cos(tf[:, :, 4])
    sc, cc = np.sin(tf[:, :, 5]), np.cos(tf[:, :, 5])

    # R = Rx(a) @ Ry(b) @ Rz(c)  (pytorch3d euler 'XYZ')
    R = np.empty((BS, P, 3, 3), f)
    R[..., 0, 0] = cb * cc
    R[..., 0, 1] = -cb * sc
    R[..., 0, 2] = sb
    R[..., 1, 0] = ca * sc + sa * sb * cc
    R[..., 1, 1] = ca * cc - sa * sb * sc
    R[..., 1, 2] = -sa * cb
    R[..., 2, 0] = sa * sc - ca * sb * cc
    R[..., 2, 1] = sa * cc + ca * sb * sc
    R[..., 2, 2] = ca * cb

    Rws = R * (w * scl[:, None])[..., None, None]    # [bs,p,i,j]
    tsum = (w[..., None] * t).sum(axis=1)            # [bs,3]
    A = Rws.sum(axis=1)                              # [bs,i,j]
    O = np.asarray(prototype_offsets, f)             # [p,v,j]
    Bv = np.asarray(base_verts, f)                   # [v,j]

    Oq = q8(O)
    Ah = q8(A); Al = q8(A - Ah.astype(f))
    Bh = q8(Bv); Bl = q8(Bv - Bh.astype(f))
    th = q8(tsum); tl = q8(tsum - th.astype(f))

    # per-slot lhs [NSLOT, 3(i), BS] and rhs [NSLOT, VPAD] fp8
    lhs = np.zeros((NSLOT, 3, BS), f8np)
    rhs = np.zeros((NSLOT, VPAD), f8np)
    # slots 0..191: ws*R x offsets, c = p*3 + j
    c = np.arange(192)
    pp, jj = c // 3, c % 3
    lhs[:192] = q8(Rws[:, pp, :, jj]).transpose(0, 2, 1)      # [c,bs,i]->[c,i,bs]
    rhs[:192, :V] = Oq[pp, :, jj]
    # slots 192/193: translation hi/lo x ones
    ones = np.ones(VPAD, f8np)
    lhs[192] = th.T; rhs[192] = ones
    lhs[193] = tl.T; rhs[193] = ones
    # slots 194..205: base term hi/lo cross products
    for j in range(3):
        for qi, (av, bv) in enumerate(((Ah, Bh), (Ah, Bl), (Al, Bh), (Al, Bl))):
            s = 194 + 4 * j + qi
            lhs[s] = av[:, :, j].T
            rhs[s, :V] = bv[:, j]

    # device layout: slot c -> (k = c % K, t = c // K)
    # chunk1 row k: [t: D(321) | W0(128) | W1(128) | pad]x2
    # chunk2 row k: [t: W2 | W3 | W4 | W5]x2
    lhs_u8 = lhs.view(np.uint8)
    rhs_u8 = rhs.view(np.uint8)
    kk = np.arange(NSLOT) % K
    tt = np.arange(NSLOT) // K
    row1 = np.zeros((128, 2, T1COLS), np.uint8)
    row2 = np.zeros((128, 2, T2COLS), np.uint8)
    for g, (i, h) in enumerate(GROUPS):
        dst, col = (row1, VC + g * 128) if g < 2 else (row2, (g - 2) * 128)
        dst[kk, tt, col:col + 128] = lhs_u8[:, i, h * 128:(h + 1) * 128]

    in2 = row2.reshape(128, LINE2)
    in_maps = []
    for core in range(N_CORES):
        r1 = row1.copy()
        r1[kk, tt, 0:VC] = rhs_u8[:, core * VC:(core + 1) * VC]
        in_maps.append({"in1": r1.reshape(128, LINE1), "in2": in2})
    return in_maps


def kernel(scales, transforms, prototype_weights, prototype_offsets, base_verts):
    nc = _get_nc()
    in_maps = _prep_inputs(
        scales, transforms, prototype_weights, prototype_offsets, base_verts)
    res = run_bass_kernel_spmd(nc, in_maps, list(range(N_CORES)))
    full = np.empty((BS, VPAD, 3), np.float32)
    for c in range(N_CORES):
        o = (np.asarray(res.results[c]["out"]).astype(np.float32) - 128.0) / QS
        vs = slice(c * VC, (c + 1) * VC)
        for g, (i, h) in enumerate(GROUPS):
            full[h * 128:(h + 1) * 128, vs, i] = o[:, g * VC:(g + 1) * VC]
    return np.ascontiguousarray(full[:, :V, :])


# revision 29
# speedup vs baseline: 1.0794x; 1.0794x over previous
"""Trainium2 Bass kernel for nn_MeshTransformer (8-core SPMD, V-sharded).

Computes, for each of BS=256 (b,s) pairs:
    out[bs, v, i] = sum_{p,j} ws[bs,p] * R[i,j](bs,p) * deformed[p,v,j]
                    + sum_p w[bs,p] * t[bs,p,i]
with R the XYZ-euler rotation, ws = w * scale, deformed = base + offsets.

Mapping (vs. the fp16 predecessor: 4x less PE work via fp8 DoubleRow):
  - Vertex dim V (2562, padded to 2568) is sharded 8 ways (321/core).
  - The whole per-(bs,v,i) contraction runs as ONE fp8e4m3 DoubleRow matmul
    per (i-plane, bs-half) group: 206 contraction slots packed 2-per-partition
    into K=103:
      * 192 slots: ws*R[i,j] x offsets[p,v,j]   (fp8 error only touches the
        small offsets term, |O| ~ 0.1)
      * 12 slots: hi/lo error-compensated split of A[i,j]=sum_p ws*R (lhs) x
        base_verts[v,j] (rhs) - the large rank-3 base term at ~fp16 accuracy
      * 2 slots: hi/lo split of the translation sum against a ones-row
    Host does all weight math in fp64 and ships ready fp8 bytes; matmul
    abs err ~0.14 vs absmax 34 (rel ~4e-3 with the uint8 output, budget 2e-2).
  - 6 groups x 2 N-chunks (161+160 <= 256 moving limit) = 12 matmuls,
    963 PE cycles total at 0.5 cycles/row; queue-stuffed so dispatch lands
    past the 3us p-state ramp and runs at the full 2.4 GHz clock.
  - Input: [D | W0 | W1] on SP HWDGE plus [W2..W5] on Pool direct SWDGE
    (desc-gen on the idle Pool engine; its transfer queues right behind
    chunk 1). Only the 103 data-carrying partition rows are moved.
  - Drains quantize PSUM->uint8 (out*QS+128) on ACT & DVE: an early single
    each (DVE, the slower engine, takes g0, the earliest), then a two-bank
    pair each ([128,2,321] in one op), balanced to finish within ~20ns.
  - Output: two SP HWDGE copies (singles' columns early, pair columns after
    the last drain) with engine-drain sem handshakes; host dequantizes.
"""

import numpy as np
from contextlib import ExitStack

import concourse.bass as bass
import concourse.tile as tile
from concourse import bacc, mybir
from concourse.bass_utils import run_bass_kernel_spmd

B, S, P, V = 16, 16, 64, 2562
BS = B * S              # 256
N_CORES = 8
VPAD = 2568             # minimal multiple of 8 covering V
VC = VPAD // N_CORES    # 321 vertices per core

K = 103                 # contraction partitions (206 slots / 2 per partition)
NSLOT = 206
T1COLS = 592            # chunk1 per-k-tile cols: 321 D | W0 W1 | pad (16B-aligned stride for dual-fp8 ldweights)
T2COLS = 512            # chunk2 per-k-tile cols: W2..W5
LINE1 = 2 * T1COLS      # 1280 B/row
LINE2 = 2 * T2COLS      # 1024 B/row
OUTC = 6 * VC           # 1926 osb columns
QS = 3.6                # uint8 output quantization scale (|out| <= ~35)

F32 = mybir.dt.float32
F16 = mybir.dt.float16
F8 = mybir.dt.float8e4
U8 = mybir.dt.uint8

GROUPS = [(0, 0), (1, 0), (2, 0), (0, 1), (1, 1), (2, 1)]  # (i, h) per g


def _build_kernel():
    nc = bacc.Bacc("TRN2", target_bir_lowering=False, debug=False)

    in1_d = nc.dram_tensor("in1", [128, LINE1], U8, kind="ExternalInput").ap()
    in2_d = nc.dram_tensor("in2", [128, LINE2], U8, kind="ExternalInput").ap()
    out_d = nc.dram_tensor("out", [128, OUTC], U8, kind="ExternalOutput").ap()

    with tile.TileContext(nc) as tc, ExitStack() as ctx:
        pool = ctx.enter_context(tc.tile_pool(name="work", bufs=1))
        psum = ctx.enter_context(tc.tile_pool(name="psum", bufs=1, space="PSUM"))
        psumw = ctx.enter_context(tc.tile_pool(name="psumw", bufs=1, space="PSUM"))

        # engine warmups: ACT function-table preload + PE p-state warm-up;
        # the bias tile feeds the ACT drains (quantization offset +128)
        dummy = pool.tile([128, 1], F16, tag="dummy")
        biasq = pool.tile([128, 1], F32, tag="biasq")
        nc.vector.memset(dummy[:], 0.25)

        # input chunk 1 (D + W0..W2) on SP HWDGE; chunk 2 (W3..W5) via Pool
        # SWDGE - its desc-gen runs on the otherwise-idle Pool engine and its
        # transfer queues right behind chunk 1, arriving ~140ns earlier than a
        # second serialized SP HWDGE chain would
        t1 = pool.tile([128, 2, T1COLS], F8, tag="t1")
        t2 = pool.tile([128, 2, T2COLS], F8, tag="t2")
        # only rows 0..K-1 carry data - don't move the zero padding rows
        nc.sync.dma_start(
            out=t1[:].bitcast(U8).rearrange("p a b -> p (a b)")[0:K, :],
            in_=in1_d[0:K, :])
        nc.gpsimd.dma_start(
            out=t2[:].bitcast(U8).rearrange("p a b -> p (a b)")[0:K, :],
            in_=in2_d[0:K, :])

        nc.gpsimd.memset(biasq[:], 128.0)
        nc.scalar.copy(dummy[:], biasq[:])
        wps = psumw.tile([1, 1], F32)
        nc.tensor.matmul(wps[:], dummy[:], dummy[:], start=True, stop=True)

        osb = pool.tile([128, OUTC], U8, tag="osb")

        # stuff the PE wait queue with no-op matmuls gated on t1 so the real
        # matmuls' seq dispatch (where the p-state is sampled) happens with the
        # sequencer continuously busy -> full clock past the ramp window
        for _ in range(2):
            nc.tensor.matmul(wps[:], t1[0:K, 0, 0:1], t1[0:K, 0, 0:1],
                             start=True, stop=True)

        # 6 groups x 2 N-chunks of fp8 DoubleRow matmuls (full contraction per
        # instruction: start=stop=True, disjoint PSUM column ranges).
        # psum tile m holds groups (2m, 2m+1) in its two banks so a pair can
        # drain as one [128, 2, VC] op.
        DR = mybir.MatmulPerfMode.DoubleRow

        def w_ap(g):
            if g < 2:
                return t1[0:K, :, VC + g * 128:VC + (g + 1) * 128]
            return t2[0:K, :, (g - 2) * 128:(g - 1) * 128]

        # separate psum tiles per drain unit: two singles (g0, g1) and two
        # 2-bank pairs ((g2,g3), (g4,g5)); a shared tile between two drains
        # makes tile-tracking serialize them cross-engine
        ptA = psum.tile([128, 512], F32, tag="ptA")
        ptB = psum.tile([128, 512], F32, tag="ptB")
        ptCD = psum.tile([128, 2, 512], F32, tag="ptCD")
        ptEF = psum.tile([128, 2, 512], F32, tag="ptEF")
        def ps_ap(g, c0, c1):
            if g == 0: return ptA[:, c0:c1]
            if g == 1: return ptB[:, c0:c1]
            pt = ptCD if g < 4 else ptEF
            return pt[:, g % 2, c0:c1]
        for g in range(6):
            for c0, c1 in ((0, 161), (161, VC)):
                nc.tensor.matmul(ps_ap(g, c0, c1), w_ap(g),
                                 t1[0:K, :, c0:c1],
                                 start=True, stop=True, perf_mode=DR)

        # drains: uint8 = psum*QS + 128; singles land first, then the pairs;
        # ACT (faster) takes the last-finishing pair
        ALU = mybir.AluOpType
        IDT = mybir.ActivationFunctionType.Identity
        ds1 = nc.alloc_semaphore("ds1")
        ds2 = nc.alloc_semaphore("ds2")
        nc.vector.tensor_scalar(osb[:, 0:VC], ptA[:, 0:VC],
                                float(QS), 128.0, op0=ALU.mult, op1=ALU.add)
        nc.scalar.activation(osb[:, VC:2 * VC], ptB[:, 0:VC], IDT,
                             bias=biasq[:], scale=float(QS))
        # engine-drain handshakes give the output DMAs a hardware-enforced
        # completion signal (belt and suspenders vs engine-tick sems)
        nc.scalar.drain().then_inc(ds1, 1)
        nc.vector.drain().then_inc(ds1, 1)
        nc.vector.tensor_scalar(osb[:, 2 * VC:4 * VC], ptCD[:, :, 0:VC],
                                float(QS), 128.0, op0=ALU.mult, op1=ALU.add)
        nc.scalar.activation(osb[:, 4 * VC:6 * VC], ptEF[:, :, 0:VC], IDT,
                             bias=biasq[:], scale=float(QS))
        nc.scalar.drain().then_inc(ds2, 1)
        nc.vector.drain().then_inc(ds2, 1)

        # output in two pieces: the singles' columns fire early, the rest
        # after the pair drains land; host dequantizes
        nc.sync.dma_start(
            out=out_d[:, 0:2 * VC], in_=osb[:, 0:2 * VC])._wait_ge(ds1, 2)
        nc.sync.dma_start(
            out=out_d[:, 2 * VC:6 * VC],
            in_=osb[:, 2 * VC:6 * VC])._wait_ge(ds2, 2)

    nc.compile()
    return nc


_NC_CACHE = None


def _get_nc():
    global _NC_CACHE
    if _NC_CACHE is None:
        _NC_CACHE = _build_kernel()
    return _NC_CACHE


def _prep_inputs(scales, transforms, prototype_weights, prototype_offsets, base_verts):
    """Host-side math (fp64) + fp8 packing; device is pure DMA+PE+drain."""
    f8np = mybir.dt.np(F8)

    def q8(x):
        return np.asarray(x, np.float32).astype(f8np)

    f = np.float64
    scl = np.asarray(scales, np.float32).reshape(BS).astype(f)
    tf = np.asarray(transforms, np.float32).reshape(BS, P, 6).astype(f)
    w = np.asarray(prototype_weights, np.float32).reshape(BS, P).astype(f)
    t = tf[:, :, 0:3]
    sa, ca = np.sin(tf[:, :, 3]), np.cos(tf[:, :, 3])
    sb, cb = np.sin(tf[:, :, 4]), np.cos(tf[:, :, 4])
    sc, cc = np.sin(tf[:, :, 5]), np.cos(tf[:, :, 5])

    # R = Rx(a) @ Ry(b) @ Rz(c)  (pytorch3d euler 'XYZ')
    R = np.empty((BS, P, 3, 3), f)
    R[..., 0, 0] = cb * cc
    R[..., 0, 1] = -cb * sc
    R[..., 0, 2] = sb
    R[..., 1, 0] = ca * sc + sa * sb * cc
    R[..., 1, 1] = ca * cc - sa * sb * sc
    R[..., 1, 2] = -sa * cb
    R[..., 2, 0] = sa * sc - ca * sb * cc
    R[..., 2, 1] = sa * cc + ca * sb * sc
    R[..., 2, 2] = ca * cb

    Rws = R * (w * scl[:, None])[..., None, None]    # [bs,p,i,j]
    tsum = (w[..., None] * t).sum(axis=1)            # [bs,3]
    A = Rws.sum(axis=1)                              # [bs,i,j]
    O = np.asarray(prototype_offsets, f)             # [p,v,j]
    Bv = np.asarray(base_verts, f)                   # [v,j]

    Oq = q8(O)
    Ah = q8(A); Al = q8(A - Ah.astype(f))
    Bh = q8(Bv); Bl = q8(Bv - Bh.astype(f))
    th = q8(tsum); tl = q8(tsum - th.astype(f))

    # per-slot lhs [NSLOT, 3(i), BS] and rhs [NSLOT, VPAD] fp8
    lhs = np.zeros((NSLOT, 3, BS), f8np)
    rhs = np.zeros((NSLOT, VPAD), f8np)
    # slots 0..191: ws*R x offsets, c = p*3 + j
    c = np.arange(192)
    pp, jj = c // 3, c % 3
    lhs[:192] = q8(Rws[:, pp, :, jj]).transpose(0, 2, 1)      # [c,bs,i]->[c,i,bs]
    rhs[:192, :V] = Oq[pp, :, jj]
    # slots 192/193: translation hi/lo x ones
    ones = np.ones(VPAD, f8np)
    lhs[192] = th.T; rhs[192] = ones
    lhs[193] = tl.T; rhs[193] = ones
    # slots 194..205: base term hi/lo cross products
    for j in range(3):
        for qi, (av, bv) in enumerate(((Ah, Bh), (Ah, Bl), (Al, Bh), (Al, Bl))):
            s = 194 + 4 * j + qi
            lhs[s] = av[:, :, j].T
            rhs[s, :V] = bv[:, j]

    # device layout: slot c -> (k = c % K, t = c // K)
    # chunk1 row k: [t: D(321) | W0(128) | W1(128) | pad]x2
    # chunk2 row k: [t: W2 | W3 | W4 | W5]x2
    lhs_u8 = lhs.view(np.uint8)
    rhs_u8 = rhs.view(np.uint8)
    kk = np.arange(NSLOT) % K
    tt = np.arange(NSLOT) // K
    row1 = np.zeros((128, 2, T1COLS), np.uint8)
    row2 = np.zeros((128, 2, T2COLS), np.uint8)
    for g, (i, h) in enumerate(GROUPS):
        dst, col = (row1, VC + g * 128) if g < 2 else (row2, (g - 2) * 128)
        dst[kk, tt, col:col + 128] = lhs_u8[:, i, h * 128:(h + 1) * 128]

    in2 = row2.reshape(128, LINE2)
    in_maps = []
    for core in range(N_CORES):
        r1 = row1.copy()
        r1[kk, tt, 0:VC] = rhs_u8[:, core * VC:(core + 1) * VC]
        in_maps.append({"in1": r1.reshape(128, LINE1), "in2": in2})
    return in_maps


def kernel(scales, transforms, prototype_weights, prototype_offsets, base_verts):
    nc = _get_nc()
    in_maps = _prep_inputs(
        scales, transforms, prototype_weights, prototype_offsets, base_verts)
    res = run_bass_kernel_spmd(nc, in_maps, list(range(N_CORES)))
    full = np.empty((BS, VPAD, 3), np.float32)
    for c in range(N_CORES):
        o = (np.asarray(res.results[c]["out"]).astype(np.float32) - 128.0) / QS
        vs = slice(c * VC, (c + 1) * VC)
        for g, (i, h) in enumerate(GROUPS):
            full[h * 128:(h + 1) * 128, vs, i] = o[:, g * VC:(g + 1) * VC]
    return np.ascontiguousarray(full[:, :V, :])
